# revision 7
# baseline (speedup 1.0000x reference)
"""CrossAttention Trainium2 kernel (8 NeuronCores, SPMD).

Sharding: batch (4) x head-group (2 groups of 8 heads) -> 8 cores.
Each core computes, for its (batch b, heads hg):
  Q^T = Wq_h @ x_q^T          [512, 1024]   (bf16, fp32 accum, +bq)
  K^T = Wk_h @ x_k^T          [512, 2048]   (bf16, fp32 accum, +bk)
  V   = x_v @ Wv_h^T          [2048, 512]   (bf16, no bias; bv folded into out bias)
  per head h:
    scores^T tiles [k,q] -> exp (bf16) -> context^T += V_h^T-slice @ exp^T   (PSUM fp32)
    scores   tiles [q,k] -> exp (fp32) + row sums -> attn = exp * (1/sum) -> HBM
  context normalized by 1/sum, out_partial = ctx_norm @ Wo_h^T + b_out -> HBM
Host: gathers attn shards, sums the two out_partials per batch.

Self-contained: hardcodes all shapes; imports only concourse + numpy.
"""

import numpy as np
import ml_dtypes

BF = ml_dtypes.bfloat16
B, LQ, LK = 4, 1024, 2048
DM, DKV = 1024, 512
NH, HD = 16, 64
HPC = 8                   # heads per core
DHC = HPC * HD            # 512 head dims per core
SCALE = HD ** -0.5        # 0.125
N_CORES = 8

_CACHE = {}


def _build():
    """Build + compile the per-core Bass module and a reusable jitted runner."""
    import jax
    import concourse.bass as bass
    import concourse.tile as tile
    from concourse import bacc, mybir
    from concourse.bass2jax import (
        install_neuronx_cc_hook,
        _bass_exec_p,
        partition_id_tensor,
    )

    f32 = mybir.dt.float32
    bf16 = mybir.dt.bfloat16
    AF = mybir.ActivationFunctionType
    ALU = mybir.AluOpType

    nc = bacc.Bacc("TRN2", target_bir_lowering=False, debug=False, num_devices=1)

    xqT_d = nc.dram_tensor("xqT", [DM, LQ], bf16, kind="ExternalInput").ap()
    xkT_d = nc.dram_tensor("xkT", [DKV, LK], bf16, kind="ExternalInput").ap()
    xvT_d = nc.dram_tensor("xvT", [DKV, LK], bf16, kind="ExternalInput").ap()
    wqT_d = nc.dram_tensor("wqT", [DM, DHC], bf16, kind="ExternalInput").ap()
    wkT_d = nc.dram_tensor("wkT", [DKV, DHC], bf16, kind="ExternalInput").ap()
    wvT_d = nc.dram_tensor("wvT", [DKV, DHC], bf16, kind="ExternalInput").ap()
    woT_d = nc.dram_tensor("woT", [DHC, DM], bf16, kind="ExternalInput").ap()
    bq_d = nc.dram_tensor("bq", [DHC], f32, kind="ExternalInput").ap()
    bk_d = nc.dram_tensor("bk", [DHC], f32, kind="ExternalInput").ap()
    bout_d = nc.dram_tensor("bout", [DM], f32, kind="ExternalInput").ap()
    ident_d = nc.dram_tensor("ident", [128, 128], f32, kind="ExternalInput").ap()
    attn_d = nc.dram_tensor("attn", [HPC, LQ, LK], f32, kind="ExternalOutput").ap()
    outp_d = nc.dram_tensor("outp", [LQ, DM], f32, kind="ExternalOutput").ap()

    from contextlib import ExitStack

    with tile.TileContext(nc) as tc, ExitStack() as ctx:
        persist = ctx.enter_context(tc.tile_pool(name="persist", bufs=1))
        smalls = ctx.enter_context(tc.tile_pool(name="smalls", bufs=2))

        # ---- persistent SBUF tensors ----
        QT = [persist.tile([128, LQ], bf16, name=f"QTs{i}") for i in range(4)]
        KT = [persist.tile([128, LK], bf16, name=f"KTs{i}") for i in range(4)]
        V = [persist.tile([128, DHC], bf16, name=f"Vs{i}") for i in range(16)]
        CTXN = [persist.tile([128, LQ], bf16, name=f"CTXNs{i}") for i in range(4)]
        WOT = [persist.tile([128, DM], bf16, name=f"WOTs{i}") for i in range(4)]
        bq_sb = persist.tile([128, 4], f32, name="bq_sb")
        bk_sb = persist.tile([128, 4], f32, name="bk_sb")
        bout_sb = persist.tile([1, DM], f32, name="bout_sb")
        id_sb = persist.tile([128, 128], f32, name="id_sb")
        ones_sb = persist.tile([1, 128], f32, name="ones_sb")
        ones128 = persist.tile([128, 64], f32, name="ones128")
        sums = persist.tile([128, 64], f32, name="sums")   # col = 16t+8hh+qi
        sum2 = persist.tile([128, 64], f32, name="sum2")   # second k-half accum
        inv = persist.tile([128, 64], f32, name="inv")

        nc.sync.dma_start(bq_sb[:, :], bq_d.rearrange("(a p) -> p a", p=128))
        nc.sync.dma_start(bk_sb[:, :], bk_d.rearrange("(a p) -> p a", p=128))
        nc.sync.dma_start(bout_sb[:, :], bout_d.rearrange("(o a) -> o a", o=1))
        nc.sync.dma_start(id_sb[:, :], ident_d[:, :])
        nc.vector.memset(ones_sb[:, :], 1.0)
        nc.vector.memset(ones128[:, :], 1.0)
        for i in range(4):
            nc.sync.dma_start(WOT[i][:, :], woT_d[128 * i:128 * (i + 1), :])

        # ---- phase 0: projections ----
        with tc.tile_pool(name="xw", bufs=1) as xw, \
             tc.tile_pool(name="ps0", bufs=4, space="PSUM") as ps0:
            wq_t = [xw.tile([128, DHC], bf16, name=f"wqt{i}") for i in range(8)]
            wk_t = [xw.tile([128, DHC], bf16, name=f"wkt{i}") for i in range(4)]
            wv_t = [xw.tile([128, DHC], bf16, name=f"wvt{i}") for i in range(4)]
            xq_t = [xw.tile([128, LQ], bf16, name=f"xqt{i}") for i in range(8)]
            xk_t = [xw.tile([128, LK], bf16, name=f"xkt{i}") for i in range(4)]
            xv_t = [xw.tile([128, LK], bf16, name=f"xvt{i}") for i in range(4)]
            for i in range(8):
                nc.sync.dma_start(wq_t[i][:, :], wqT_d[128 * i:128 * (i + 1), :])
                nc.sync.dma_start(xq_t[i][:, :], xqT_d[128 * i:128 * (i + 1), :])
            for i in range(4):
                nc.sync.dma_start(wk_t[i][:, :], wkT_d[128 * i:128 * (i + 1), :])
                nc.sync.dma_start(wv_t[i][:, :], wvT_d[128 * i:128 * (i + 1), :])
                nc.sync.dma_start(xk_t[i][:, :], xkT_d[128 * i:128 * (i + 1), :])
                nc.sync.dma_start(xv_t[i][:, :], xvT_d[128 * i:128 * (i + 1), :])

            # Q^T [512, 1024]
            for i in range(4):
                for qc in range(2):
                    psq = ps0.tile([128, 512], f32, name="psq", tag="ps0")
                    for kc in range(8):
                        nc.tensor.matmul(
                            psq[:, :], wq_t[kc][:, 128 * i:128 * (i + 1)],
                            xq_t[kc][:, 512 * qc:512 * (qc + 1)],
                            start=(kc == 0), stop=(kc == 7))
                    nc.vector.tensor_scalar_add(
                        QT[i][:, 512 * qc:512 * (qc + 1)], psq[:, :], bq_sb[:, i:i + 1])
            # K^T [512, 2048]
            for i in range(4):
                for kc2 in range(4):
                    psk = ps0.tile([128, 512], f32, name="psk", tag="ps0")
                    for kc in range(4):
                        nc.tensor.matmul(
                            psk[:, :], wk_t[kc][:, 128 * i:128 * (i + 1)],
                            xk_t[kc][:, 512 * kc2:512 * (kc2 + 1)],
                            start=(kc == 0), stop=(kc == 3))
                    nc.vector.tensor_scalar_add(
                        KT[i][:, 512 * kc2:512 * (kc2 + 1)], psk[:, :], bk_sb[:, i:i + 1])
            # V [2048, 512] (no bias)
            for ki in range(16):
                psv = ps0.tile([128, 512], f32, name="psv", tag="ps0")
                for kc in range(4):
                    nc.tensor.matmul(
                        psv[:, :], xv_t[kc][:, 128 * ki:128 * (ki + 1)],
                        wv_t[kc][:, :],
                        start=(kc == 0), stop=(kc == 3))
                nc.vector.tensor_copy(V[ki][:, :], psv[:, :])

        # ---- per head-pair ----
        for t in range(4):
            hA, hB = 2 * t, 2 * t + 1
            with tc.tile_pool(name=f"psC{t}", bufs=1, space="PSUM") as psC:
                ctx_ps = psC.tile([128, LQ], f32, name=f"ctxps{t}")
                # --- A: scores^T -> exp(bf16) -> context accum ---
                with tc.tile_pool(name=f"psT{t}", bufs=3, space="PSUM") as psT, \
                     tc.tile_pool(name=f"eT{t}", bufs=3) as eTp:
                    for ki in range(16):
                        for qh in range(2):
                            st = psT.tile([128, 1024], f32, name="st", tag="st")
                            nc.tensor.matmul(
                                st[:, 0:512], KT[t][0:64, 128 * ki:128 * (ki + 1)],
                                QT[t][0:64, 512 * qh:512 * (qh + 1)],
                                start=True, stop=True)
                            nc.tensor.matmul(
                                st[:, 512:1024], KT[t][64:128, 128 * ki:128 * (ki + 1)],
                                QT[t][64:128, 512 * qh:512 * (qh + 1)],
                                start=True, stop=True)
                            et = eTp.tile([128, 1024], bf16, name="et", tag="et")
                            nc.scalar.activation(et[:, :], st[:, :], AF.Exp, scale=SCALE)
                            # skip_group_check: the sim's PSUM zero-region
                            # tracker drops the partition base, so the two
                            # col-tiled head groups falsely alias.
                            nc.tensor.matmul(
                                ctx_ps[0:64, 512 * qh:512 * (qh + 1)],
                                V[ki][:, 64 * hA:64 * hA + 64], et[:, 0:512],
                                start=(ki == 0), stop=(ki == 15),
                                skip_group_check=True)
                            nc.tensor.matmul(
                                ctx_ps[64:128, 512 * qh:512 * (qh + 1)],
                                V[ki][:, 64 * hB:64 * hB + 64], et[:, 512:1024],
                                start=(ki == 0), stop=(ki == 15),
                                tile_position=(0, 64), skip_group_check=True)
                # --- B: scores -> exp(fp32)+sums -> attn out; ctx normalize ---
                with tc.tile_pool(name=f"psB{t}", bufs=2, space="PSUM") as psB, \
                     tc.tile_pool(name=f"psS{t}", bufs=1, space="PSUM") as psS, \
                     tc.tile_pool(name=f"eB{t}", bufs=2) as eBp, \
                     tc.tile_pool(name=f"aS{t}", bufs=3) as aSp:
                    inv_rep = psS.tile([128, LQ], f32, name=f"invrep{t}")
                    for hh in range(2):
                        h = 2 * t + hh
                        r0 = 64 * hh
                        for qi in range(8):
                            c = 16 * t + 8 * hh + qi
                            eb = eBp.tile([128, 2048], f32, name="eb", tag="eb")
                            for kh in range(2):
                                sb_ = psB.tile([128, 1024], f32, name="sb", tag="sb")
                                for kj in range(2):
                                    nc.tensor.matmul(
                                        sb_[:, 512 * kj:512 * (kj + 1)],
                                        QT[t][r0:r0 + 64, 128 * qi:128 * (qi + 1)],
                                        KT[t][r0:r0 + 64,
                                              1024 * kh + 512 * kj:1024 * kh + 512 * (kj + 1)],
                                        start=True, stop=True)
                                acc = (sums if kh == 0 else sum2)[:, c:c + 1]
                                nc.scalar.activation(
                                    eb[:, 1024 * kh:1024 * (kh + 1)], sb_[:, :],
                                    AF.Exp, scale=SCALE, accum_out=acc)
                            nc.vector.tensor_add(
                                sums[:, c:c + 1], sums[:, c:c + 1], sum2[:, c:c + 1])
                            nc.vector.reciprocal(inv[:, c:c + 1], sums[:, c:c + 1])
                            at = aSp.tile([128, 2048], f32, name="at", tag="at")
                            nc.vector.tensor_scalar_mul(at[:, :], eb[:, :], inv[:, c:c + 1])
                            nc.sync.dma_start(
                                attn_d[h, 128 * qi:128 * (qi + 1), :], at[:, :])
                    # replicate 1/sum along free dim (cols 0:64 head A,
                    # 64:128 head B), transpose into inv_rep[128 d, 128 q]
                    for qi in range(8):
                        cA = 16 * t + qi
                        cB = 16 * t + 8 + qi
                        iq = smalls.tile([128, 128], f32, name="iq", tag="iq")
                        nc.vector.tensor_scalar_mul(iq[:, 0:64], ones128[:, :],
                                                    inv[:, cA:cA + 1])
                        nc.vector.tensor_scalar_mul(iq[:, 64:128], ones128[:, :],
                                                    inv[:, cB:cB + 1])
                        nc.tensor.transpose(
                            inv_rep[:, 128 * qi:128 * (qi + 1)], iq[:, :],
                            id_sb[:, :])
                    inv_sb = smalls.tile([128, LQ], f32, name="inv_sb", tag="inv_sb")
                    nc.vector.tensor_copy(inv_sb[:, :], inv_rep[:, :])
                    nc.vector.tensor_mul(CTXN[t][:, :], ctx_ps[:, :], inv_sb[:, :])

        # ---- out-projection ----
        with tc.tile_pool(name="psO", bufs=2, space="PSUM") as psO, \
             tc.tile_pool(name="oS", bufs=3) as oSp:
            for qi in range(8):
                for n2 in range(2):
                    po = psO.tile([128, 512], f32, name="po", tag="po")
                    for kt in range(4):
                        nc.tensor.matmul(
                            po[:, :], CTXN[kt][:, 128 * qi:128 * (qi + 1)],
                            WOT[kt][:, 512 * n2:512 * (n2 + 1)],
                            start=(kt == 0), stop=False)
                    nc.tensor.matmul(
                        po[:, :], ones_sb[0:1, 0:128],
                        bout_sb[0:1, 512 * n2:512 * (n2 + 1)],
                        start=False, stop=True)
                    ot = oSp.tile([128, 512], f32, name="ot", tag="ot")
                    nc.vector.tensor_copy(ot[:, :], po[:, :])
                    nc.sync.dma_start(
                        outp_d[128 * qi:128 * (qi + 1), 512 * n2:512 * (n2 + 1)],
                        ot[:, :])

    nc.compile()

    # ---- reusable jitted PJRT runner (mirrors bass2jax.run_bass_via_pjrt) ----
    install_neuronx_cc_hook()
    in_names, out_names, out_avals, zero_outs = [], [], [], []
    pname = nc.partition_id_tensor.name if nc.partition_id_tensor else None
    for alloc in nc.m.functions[0].allocations:
        if not isinstance(alloc, mybir.MemoryLocationSet):
            continue
        name = alloc.memorylocations[0].name
        if alloc.kind == "ExternalInput":
            if name != pname:
                in_names.append(name)
        elif alloc.kind == "ExternalOutput":
            out_names.append(name)
            shape = tuple(alloc.tensor_shape)
            dtype = mybir.dt.np(alloc.dtype)
            out_avals.append(jax.core.ShapedArray(shape, dtype))
            zero_outs.append(np.zeros(shape, dtype))
    n_params = len(in_names)
    all_in = in_names + out_names + ([pname] if pname else [])

    def _body(*args):
        ops = list(args)
        if pname:
            ops.append(partition_id_tensor())
        return tuple(_bass_exec_p.bind(
            *ops, out_avals=tuple(out_avals), in_names=tuple(all_in),
            out_names=tuple(out_names), lowering_input_output_aliases=(),
            sim_require_finite=True, sim_require_nnan=True, nc=nc))

    from jax.sharding import Mesh, PartitionSpec
    from jax.experimental.shard_map import shard_map

    devices = jax.devices()[:N_CORES]
    mesh = Mesh(np.asarray(devices), ("core",))
    n_outs = len(out_names)
    sharded = jax.jit(
        shard_map(_body, mesh=mesh,
                  in_specs=(PartitionSpec("core"),) * (n_params + n_outs),
                  out_specs=(PartitionSpec("core"),) * n_outs,
                  check_rep=False),
        keep_unused=True)

    def run(per_core_inmaps):
        concat_in = [
            np.concatenate([np.asarray(per_core_inmaps[c][n]) for c in range(N_CORES)],
                           axis=0)
            for n in in_names[:n_params]]
        concat_zeros = [
            np.zeros((N_CORES * z.shape[0], *z.shape[1:]), z.dtype) for z in zero_outs]
        out_arrs = sharded(*concat_in, *concat_zeros)
        return [
            {n: np.asarray(out_arrs[i]).reshape(N_CORES, *out_avals[i].shape)[c]
             for i, n in enumerate(out_names)}
            for c in range(N_CORES)]

    return {"nc": nc, "run": run, "in_names": in_names[:n_params],
            "sharded": sharded, "out_names": out_names, "out_avals": out_avals}


def _prep_core_inputs(query, key, value, Wq, bq, Wk, bk, Wv, bv, Wo, bo):
    """Per-core input dict (sharding + layout + dtype prep)."""
    ident = np.eye(128, dtype=np.float32)
    per_core = []
    for c in range(N_CORES):
        b, g = c // 2, c % 2
        hs = slice(DHC * g, DHC * (g + 1))       # this core's 512 head dims
        wq_h = Wq[hs, :]                          # [512, 1024]
        wk_h = Wk[hs, :]                          # [512, 512]
        wv_h = Wv[hs, :]                          # [512, 512]
        wo_h = Wo[:, hs]                          # [1024, 512]
        bv_h = bv[hs]                             # [512]
        b_out = bv_h.astype(np.float64) @ wo_h.T.astype(np.float64)
        b_out = b_out.astype(np.float32)
        if g == 0:
            b_out = b_out + bo
        per_core.append({
            "xqT": np.ascontiguousarray(query[b].T).astype(BF),
            "xkT": np.ascontiguousarray(key[b].T).astype(BF),
            "xvT": np.ascontiguousarray(value[b].T).astype(BF),
            "wqT": np.ascontiguousarray(wq_h.T).astype(BF),
            "wkT": np.ascontiguousarray(wk_h.T).astype(BF),
            "wvT": np.ascontiguousarray(wv_h.T).astype(BF),
            "woT": np.ascontiguousarray(wo_h.T).astype(BF),
            "bq": bq[hs].astype(np.float32),
            "bk": bk[hs].astype(np.float32),
            "bout": b_out,
            "ident": ident,
        })
    return per_core


def kernel(query, key, value, Wq, bq, Wk, bk, Wv, bv, Wo, bo):
    query = np.asarray(query, dtype=np.float32)
    key = np.asarray(key, dtype=np.float32)
    value = np.asarray(value, dtype=np.float32)
    Wq, bq = np.asarray(Wq, np.float32), np.asarray(bq, np.float32)
    Wk, bk = np.asarray(Wk, np.float32), np.asarray(bk, np.float32)
    Wv, bv = np.asarray(Wv, np.float32), np.asarray(bv, np.float32)
    Wo, bo = np.asarray(Wo, np.float32), np.asarray(bo, np.float32)

    if "mod" not in _CACHE:
        _CACHE["mod"] = _build()
    mod = _CACHE["mod"]

    per_core = _prep_core_inputs(query, key, value, Wq, bq, Wk, bk, Wv, bv, Wo, bo)
    results = mod["run"](per_core)

    attn = np.empty((B, NH, LQ, LK), np.float32)
    output = np.empty((B, LQ, DM), np.float32)
    for b in range(B):
        attn[b, 0:HPC] = results[2 * b]["attn"]
        attn[b, HPC:NH] = results[2 * b + 1]["attn"]
        output[b] = results[2 * b]["outp"] + results[2 * b + 1]["outp"]
    return (output, attn)
